# revision 24
# baseline (speedup 1.0000x reference)
"""3-layer MLP (dense_mlp) Trainium2 Bass kernel.

Reference computation (fp32):
    h1  = relu(x @ w1 + b1)     x: [4096, 2048], w1: [2048, 4096]
    h2  = relu(h1 @ w2 + b2)    w2: [4096, 4096]
    out = h2 @ w3 + b3          w3: [4096, 1000]

Strategy: pure data-parallel over the batch across 8 NeuronCores (512
rows each, weights replicated, no collectives). Matmuls run in bf16:
same 215.8ns/matmul PE rate as f32r at full clock but half the
weight-DMA bytes, which removes every DMA stall (f32r needs ~300GB/s
sustained, right at the HBM ceiling). absmax rel err is ~4.5e-3, well
under the 2e-2 gate. fp8 was measured at 6.4e-2 for even a single
layer — unusable; DoubleRow fp8 with error compensation costs as many
PE passes as bf16, so there is no faster correct dtype.

Inside a core the activations live in transposed [feature, batch]
layout so each layer is psum[f, b] += W[k, f].T @ actT[k, b]: the
weight tile is the stationary operand and the bias is a per-partition
scalar folded into the ScalarE relu(psum + b) evaluation. The host
pre-transposes x / post-transposes the logits (cheap numpy).

Weights are pre-packed so each block DMA is [128, ks, fw] with
ks*fw*2B contiguous per partition (8KB for layers 2/3; layer 1 uses
smaller 4KB blocks so the stream start paces finely against the
concurrent x load):
w_packed[kk, g, p, s, :] = W[(ks*kk+s)*128 + p, g*fw : (g+1)*fw].

PSUM groups are 4 banks wide (FW=512) so group N+1 fills bank set B
while group N's activations drain bank set A — no boundary stall.
Layer 3 tapers its groups [4, 2, 1, 1] f-tiles (each packed so blocks
stay full-line) so the bias-add + store epilogue of each group hides
under the next group's matmuls; only the last 128-column group's
epilogue is exposed (~2.9us including the fixed queue-drain chain).

The first weight block of layer 1 is additionally fetched as KS_L1
separate 128KB chunks so the first real matmul only waits for one
small transfer; warmup matmuls on a gpsimd-memset tile keep the PE
ramping from engine bring-up (~7.6us) until the stream starts
(~10.5us), so the clock gate is 8/8 when real data flows.

Measured on 8 axon trn2 cores: 405.3us (PE stream floor is 1792
matmuls x 215.8ns = 386.7us; head+tail+ramp make up the rest).
"""

import os

import numpy as np
import ml_dtypes

import concourse.bass as bass
import concourse.mybir as mybir
import concourse.tile as tile
from concourse import bacc
from concourse.bass_utils import run_bass_kernel_spmd

P = 128
N_CORES = 8
B_TOTAL = 4096
B = B_TOTAL // N_CORES  # per-core batch rows
D0, D1, D2 = 2048, 4096, 4096
D3_RAW, D3 = 1000, 1024  # classifier dim padded to a multiple of 128

FW = 512        # f-columns per psum group (4 tiles -> 4 psum banks)
FGL = FW // P   # f-tiles per group = 4


def _ks(mode):
    # K-tiles per weight DMA block for layers 2/3 (16KB/partition lines
    # in f32, 8KB in bf16). Layer 1 uses smaller blocks (below) so the
    # stream start is finely paced against the concurrent x load.
    return 8


KS_L1 = 4
L3_GROUPS = [4, 2, 1, 1]  # tapered f-tile group sizes for layer 3

f32 = mybir.dt.float32
bf16 = mybir.dt.bfloat16


def _act_dt(mode):
    if mode == "bf16":
        return bf16
    if mode == "f32r":
        return mybir.dt.float32r
    return f32


def build_nc(mode: str = "bf16") -> bass.Bass:
    KS = _ks(mode)
    K0, K1, K2 = D0 // P, D1 // P, D2 // P
    F1, F2, F3 = D1 // P, D2 // P, D3 // P
    G1, G2 = F1 // FGL, F2 // FGL
    act_dt = _act_dt(mode)

    nc = bacc.Bacc("TRN2", target_bir_lowering=False, name="mlp3")
    xT = nc.dram_tensor("xT", [P, K0, B], act_dt, kind="ExternalInput")
    w1 = nc.dram_tensor("w1", [K0 // KS_L1, G1, P, KS_L1, FW], act_dt,
                        kind="ExternalInput")
    b1 = nc.dram_tensor("b1", [P, F1], f32, kind="ExternalInput")
    w2 = nc.dram_tensor("w2", [K1 // KS, G2, P, KS, FW], act_dt,
                        kind="ExternalInput")
    b2 = nc.dram_tensor("b2", [P, F2], f32, kind="ExternalInput")
    w3g = [
        nc.dram_tensor(
            f"w3g{gi}",
            [K2 // (KS * FGL // fgl), P, KS * FGL // fgl, fgl * P],
            act_dt, kind="ExternalInput")
        for gi, fgl in enumerate(L3_GROUPS)
    ]
    b3 = nc.dram_tensor("b3", [P, F3], f32, kind="ExternalInput")
    out = nc.dram_tensor("out", [P, F3, B], f32, kind="ExternalOutput")

    with tile.TileContext(nc) as tc:
        consts = tc.alloc_tile_pool(name="consts", bufs=1, side="left")
        b1_sb = consts.tile([P, F1], f32, name="b1_sb")
        b2_sb = consts.tile([P, F2], f32, name="b2_sb")
        b3_sb = consts.tile([P, F3], f32, name="b3_sb")
        # warmup operand: memset on gpsimd (whose queue comes up
        # earliest, ~6us) so the PE can start ramping right at engine
        # bring-up. f32r is not a memset-able dtype; plain f32 warm
        # matmuls warm the PE just as well.
        warm_dt = bf16 if mode == "bf16" else f32
        warm = consts.tile([P, B], warm_dt, name="warm")
        nc.gpsimd.memset(warm, 1.0)

        # biases ride the gpsimd SWDGE: tiny, not needed until ~30us,
        # keeps the scalar queue free to fire the x chunks immediately
        # (and they queue behind the warm-tile memset above)
        nc.gpsimd.dma_start(b1_sb, b1[:, :])
        nc.gpsimd.dma_start(b2_sb, b2[:, :])
        nc.gpsimd.dma_start(b3_sb, b3[:, :])

        p_xT = tc.alloc_tile_pool(name="xT", bufs=1, side="left")
        xT_sb = p_xT.tile([P, K0, B], act_dt, name="xT_sb")
        # chunk the input load per k-tile (ACT HWDGE ring, so the
        # weight stream on the SP ring is not delayed behind it).
        # The last 4 k-tiles are not consumed until ~24us, so they are
        # issued later (below) on the SP ring, behind the layer-1 g0
        # weight blocks — thinning HBM contention in the critical
        # 9-18us window where g0's blocks land just-in-time.
        X_EARLY = K0 - 4
        for k in range(X_EARLY):
            nc.scalar.dma_start(xT_sb[:, k, :], xT[:, k, :])

        # first-block chunks: layer 1 / group 0 / kk 0 arrives as KS_L1
        # separate 128KB transfers on the SP ring (whose queue fires
        # first, ~6.8us) so the first real matmul only waits for one
        # small transfer. NB measured: routing these via the scalar
        # queue instead costs ~4us (its main-block+ACT-table preamble
        # delays the triggers); extending chunking to kk=1 also loses
        # (trigger latency pushes later block triggers out).
        wfirst = tc.alloc_tile_pool(name="wfirst", bufs=KS_L1,
                                    side="left")
        chunks = [wfirst.tile([P, FW], act_dt, name=f"wc{s}", tag="wc")
                  for s in range(KS_L1)]
        for s in range(KS_L1):
            nc.sync.dma_start(chunks[s], w1[0, 0, :, s, :])
        for k in range(X_EARLY, K0):
            nc.sync.dma_start(xT_sb[:, k, :], xT[:, k, :])

        wpool = tc.alloc_tile_pool(
            name="w", bufs=5 if mode == "bf16" else 3, side="right")
        mmps = tc.alloc_tile_pool(name="mmpsum", bufs=8, space="PSUM")

        # HAM warmup: keep the PE busy from ~2us until the first weight
        # chunk lands, so the clock gate is 8/8 when the real stream
        # starts
        # sized so warmup ends right as the first weight chunk + x
        # chunk land (~10-12us): the PE ramp continues into real MMs
        warm_n = 7 if mode == "bf16" else 6
        wps = mmps.tile([P, B], f32, name="wps", tag="ps")
        for i in range(warm_n):
            nc.tensor.matmul(wps, warm[:, :P], warm,
                             start=(i == 0), stop=(i == warm_n - 1))

        def layer(actT, bias_sb, outT, n_k, groups, relu, wsrc,
                  store_to=None, spool=None, first_chunks=False):
            """groups: list of (fa, fgl, ksg). wsrc(gi, kk) -> dram
            block AP [P, ksg, fgl*P] (always ksg*fgl*P = KS*FW elements
            so every wt slot is one 2MB 16KB-line transfer).
            first_chunks: group 0 / kk 0 reads the pre-split chunk
            tiles instead."""
            for gi, (fa, fgl, ksg) in enumerate(groups):
                fw = fgl * P
                psums = [
                    mmps.tile([P, B], f32, name=f"ps{fa + f}", tag="ps")
                    for f in range(fgl)
                ]
                for kk in range(n_k // ksg):
                    use_chunks = first_chunks and gi == 0 and kk == 0
                    if not use_chunks:
                        wt = wpool.tile([P, ksg, fw], act_dt, name="wt",
                                        tag="wt")
                        nc.sync.dma_start(wt, wsrc(gi, kk))
                    for s in range(ksg):
                        k = kk * ksg + s
                        for f in range(fgl):
                            stat = (chunks[s][:, f * P:(f + 1) * P]
                                    if use_chunks else
                                    wt[:, s, f * P:(f + 1) * P])
                            nc.tensor.matmul(
                                psums[f],
                                stat,
                                actT[:, k, :],
                                start=(k == 0),
                                stop=(k == n_k - 1),
                            )
                if relu:
                    for f in range(fgl):
                        fi = fa + f
                        nc.scalar.activation(
                            outT[:, fi, :],
                            psums[f],
                            mybir.ActivationFunctionType.Relu,
                            bias=bias_sb[:, fi:fi + 1],
                            scale=1.0,
                        )
                else:
                    # bias-add on ScalarE (idle once the relus are done)
                    # so the store trigger on the same queue follows with
                    # no cross-engine semaphore hop
                    ot = spool.tile([P, FGL, B], f32, name="ot", tag="ot")
                    for f in range(fgl):
                        fi = fa + f
                        nc.scalar.activation(
                            ot[:, f, :],
                            psums[f],
                            mybir.ActivationFunctionType.Identity,
                            bias=bias_sb[:, fi:fi + 1],
                            scale=1.0,
                        )
                    nc.scalar.dma_start(
                        store_to[:, fa:fa + fgl, :], ot[:, :fgl, :])

        def uniform_groups(n_f, ks):
            return [(g * FGL, FGL, ks) for g in range(n_f // FGL)]

        p_h1 = tc.alloc_tile_pool(name="h1", bufs=1, side="right")
        h1T = p_h1.tile([P, K1, B], act_dt, name="h1T")
        layer(xT_sb, b1_sb, h1T, K0, uniform_groups(F1, KS_L1), True,
              lambda gi, kk: w1[kk, gi], first_chunks=True)
        wfirst.release()
        p_xT.release()

        p_h2 = tc.alloc_tile_pool(name="h2", bufs=1, side="left")
        h2T = p_h2.tile([P, K2, B], act_dt, name="h2T")
        layer(h1T, b2_sb, h2T, K1, uniform_groups(F2, KS), True,
              lambda gi, kk: w2[kk, gi])
        p_h1.release()

        # every layer-3 block stays a full 16KB-line transfer: narrow
        # f groups pack more k-tiles per block (ksg = KS*FGL/fgl)
        l3_groups = []
        fa = 0
        for fgl in L3_GROUPS:
            l3_groups.append((fa, fgl, KS * FGL // fgl))
            fa += fgl
        p_oT = tc.alloc_tile_pool(name="oT", bufs=2, side="right")
        layer(h2T, b3_sb, None, K2, l3_groups, False,
              lambda gi, kk: w3g[gi][kk],
              store_to=out, spool=p_oT)
        p_h2.release()
        mmps.release()
        p_oT.release()
        wpool.release()
        consts.release()
    nc.compile()
    return nc


def _pack_w(w: np.ndarray, np_dt, fw, ks) -> np.ndarray:
    """[d_in, d_out] -> [K/ks, d_out/fw, P, ks, fw] so one [128, ks, fw]
    DMA block reads ks*fw*itemsize bytes contiguous per partition."""
    d_in, d_out = w.shape
    K, G = d_in // P, d_out // fw
    v = w.reshape(K // ks, ks, P, G, fw)
    return np.ascontiguousarray(v.transpose(0, 3, 2, 1, 4)).astype(np_dt)


def _pack_w3_group(w: np.ndarray, np_dt, ks) -> np.ndarray:
    """[d_in, fw] -> [K/ks, P, ks, fw] (single column group)."""
    d_in, fw = w.shape
    K = d_in // P
    v = w.reshape(K // ks, ks, P, fw)
    return np.ascontiguousarray(v.transpose(0, 2, 1, 3)).astype(np_dt)


LAST_RESULT = None  # BassKernelResults of the most recent run (for test.py)


def _ensure_axon_ntff_hook():
    """Register the NTFF-profile hook that bass_utils expects under axon.
    The agent image's antenv lacks axon_hooks; synthesize it from the
    slim ctypes shim in trn_agent_boot. Only needed for trace runs."""
    import sys
    import types

    try:
        from antenv.axon_hooks import get_axon_ntff_profile_hook  # noqa: F401
        return
    except ImportError:
        pass
    try:
        import antenv
        from trn_agent_boot.trn_boot import _ntff_profile_via_ctypes

        hook = _ntff_profile_via_ctypes("/opt/axon/libaxon_pjrt.so")
        mod = types.ModuleType("antenv.axon_hooks")
        state = {"hook": hook}
        mod.get_axon_ntff_profile_hook = lambda: state["hook"]
        mod.set_axon_ntff_profile_hook = lambda h: state.update(hook=h)
        sys.modules["antenv.axon_hooks"] = mod
        antenv.axon_hooks = mod
    except Exception as e:  # degrade to untraced run
        print(f"ntff hook setup failed ({e!r}); tracing disabled")


def kernel(x, w1, b1, w2, b2, w3, b3):
    global LAST_RESULT
    os.environ.setdefault("JAX_PLATFORMS", "axon")
    mode = os.environ.get("KERNEL_MM_MODE", "bf16")
    trace = os.environ.get("KERNEL_TRACE", "0") == "1"
    if trace:
        _ensure_axon_ntff_hook()

    x = np.asarray(x, dtype=np.float32)
    b1 = np.asarray(b1, dtype=np.float32)
    b2 = np.asarray(b2, dtype=np.float32)
    b3 = np.asarray(b3, dtype=np.float32)

    w3f = np.zeros((D2, D3), dtype=np.float32)
    w3f[:, :D3_RAW] = np.asarray(w3, dtype=np.float32)
    b3f = np.zeros((D3,), dtype=np.float32)
    b3f[:D3_RAW] = b3

    np_dt = ml_dtypes.bfloat16 if mode == "bf16" else np.float32
    KS = _ks(mode)
    w1p = _pack_w(np.asarray(w1, dtype=np.float32), np_dt, FW, KS_L1)
    w2p = _pack_w(np.asarray(w2, dtype=np.float32), np_dt, FW, KS)
    w3ps = {}
    fa = 0
    for gi, fgl in enumerate(L3_GROUPS):
        fw = fgl * P
        w3ps[f"w3g{gi}"] = _pack_w3_group(
            w3f[:, fa * P:fa * P + fw], np_dt, KS * FGL // fgl)
        fa += fgl
    b1p = np.ascontiguousarray(b1.reshape(D1 // P, P).T)
    b2p = np.ascontiguousarray(b2.reshape(D2 // P, P).T)
    b3p = np.ascontiguousarray(b3f.reshape(D3 // P, P).T)

    nc = build_nc(mode=mode)
    K0 = D0 // P
    in_maps = []
    for c in range(N_CORES):
        xs = x[c * B:(c + 1) * B]  # [B, D0]
        # xT[p, k, b] = x[b, k*128 + p]
        xT = np.ascontiguousarray(
            xs.reshape(B, K0, P).transpose(2, 1, 0)).astype(np_dt)
        in_maps.append({
            "xT": xT,
            "w1": w1p, "b1": b1p,
            "w2": w2p, "b2": b2p,
            "b3": b3p,
            **w3ps,
        })

    res = run_bass_kernel_spmd(
        nc, in_maps, core_ids=list(range(N_CORES)), trace=trace
    )
    LAST_RESULT = res
    outs = []
    for r in res.results:
        oT = r["out"]  # [P, F3, B]; logits[b, fg*128+p] = oT[p, fg, b]
        outs.append(oT.transpose(2, 1, 0).reshape(B, D3))
    out = np.concatenate(outs, axis=0)
    return np.ascontiguousarray(out[:, :D3_RAW].astype(np.float32))


# revision 25
# speedup vs baseline: 1.0126x; 1.0126x over previous
"""3-layer MLP (dense_mlp) Trainium2 Bass kernel.

Reference computation (fp32):
    h1  = relu(x @ w1 + b1)     x: [4096, 2048], w1: [2048, 4096]
    h2  = relu(h1 @ w2 + b2)    w2: [4096, 4096]
    out = h2 @ w3 + b3          w3: [4096, 1000]

Strategy: pure data-parallel over the batch across 8 NeuronCores (512
rows each, weights replicated, no collectives). Matmuls run in bf16:
same 215.8ns/matmul PE rate as f32r at full clock but half the
weight-DMA bytes, which removes every DMA stall (f32r needs ~300GB/s
sustained, right at the HBM ceiling). absmax rel err is ~4.5e-3, well
under the 2e-2 gate. fp8 was measured at 6.4e-2 for even a single
layer — unusable; DoubleRow fp8 with error compensation costs as many
PE passes as bf16, so there is no faster correct dtype.

Inside a core the activations live in transposed [feature, batch]
layout so each layer is psum[f, b] += W[k, f].T @ actT[k, b]: the
weight tile is the stationary operand and the bias is a per-partition
scalar folded into the ScalarE relu(psum + b) evaluation. The host
pre-transposes x / post-transposes the logits (cheap numpy).

Weights are pre-packed so each block DMA is [128, ks, fw] with
ks*fw*2B contiguous per partition (8KB for layers 2/3; layer 1 uses
smaller 4KB blocks so the stream start paces finely against the
concurrent x load):
w_packed[kk, g, p, s, :] = W[(ks*kk+s)*128 + p, g*fw : (g+1)*fw].

PSUM groups are 4 banks wide (FW=512) so group N+1 fills bank set B
while group N's activations drain bank set A — no boundary stall.
Layer 3 tapers its groups [4, 2, 1, 1] f-tiles (each packed so blocks
stay full-line) so the bias-add + store epilogue of each group hides
under the next group's matmuls; only the last 128-column group's
epilogue is exposed (~2.9us including the fixed queue-drain chain).

The first weight block of layer 1 is additionally fetched as KS_L1
separate 128KB chunks so the first real matmul only waits for one
small transfer; warmup matmuls on a gpsimd-memset tile keep the PE
ramping from engine bring-up (~7.6us) until the stream starts
(~10.5us), so the clock gate is 8/8 when real data flows.

Measured on 8 axon trn2 cores: 405.3us (PE stream floor is 1792
matmuls x 215.8ns = 386.7us; head+tail+ramp make up the rest).
"""

import os

import numpy as np
import ml_dtypes

import concourse.bass as bass
import concourse.mybir as mybir
import concourse.tile as tile
from concourse import bacc
from concourse.bass_utils import run_bass_kernel_spmd

P = 128
N_CORES = 8
B_TOTAL = 4096
B = B_TOTAL // N_CORES  # per-core batch rows
D0, D1, D2 = 2048, 4096, 4096
D3_RAW, D3 = 1000, 1024  # classifier dim padded to a multiple of 128

FW = 512        # f-columns per psum group (4 tiles -> 4 psum banks)
FGL = FW // P   # f-tiles per group = 4


def _ks(mode):
    # K-tiles per weight DMA block for layers 2/3 (16KB/partition lines
    # in f32, 8KB in bf16). Layer 1 uses smaller blocks (below) so the
    # stream start is finely paced against the concurrent x load.
    return 8


KS_L1 = 4
L3_GROUPS = [4, 2, 1, 1]  # tapered f-tile group sizes for layer 3

f32 = mybir.dt.float32
bf16 = mybir.dt.bfloat16


def _act_dt(mode):
    if mode == "bf16":
        return bf16
    if mode == "f32r":
        return mybir.dt.float32r
    return f32


def build_nc(mode: str = "bf16") -> bass.Bass:
    KS = _ks(mode)
    K0, K1, K2 = D0 // P, D1 // P, D2 // P
    F1, F2, F3 = D1 // P, D2 // P, D3 // P
    G1, G2 = F1 // FGL, F2 // FGL
    act_dt = _act_dt(mode)

    nc = bacc.Bacc("TRN2", target_bir_lowering=False, name="mlp3")
    xT = nc.dram_tensor("xT", [P, K0, B], act_dt, kind="ExternalInput")
    w1 = nc.dram_tensor("w1", [K0 // KS_L1, G1, P, KS_L1, FW], act_dt,
                        kind="ExternalInput")
    b1 = nc.dram_tensor("b1", [P, F1], f32, kind="ExternalInput")
    w2 = nc.dram_tensor("w2", [K1 // KS, G2, P, KS, FW], act_dt,
                        kind="ExternalInput")
    b2 = nc.dram_tensor("b2", [P, F2], f32, kind="ExternalInput")
    w3g = [
        nc.dram_tensor(
            f"w3g{gi}",
            [K2 // (KS * FGL // fgl), P, KS * FGL // fgl, fgl * P],
            act_dt, kind="ExternalInput")
        for gi, fgl in enumerate(L3_GROUPS)
    ]
    b3 = nc.dram_tensor("b3", [P, F3], f32, kind="ExternalInput")
    out = nc.dram_tensor("out", [P, F3, B], f32, kind="ExternalOutput")

    with tile.TileContext(nc) as tc:
        consts = tc.alloc_tile_pool(name="consts", bufs=1, side="left")
        b1_sb = consts.tile([P, F1], f32, name="b1_sb")
        b2_sb = consts.tile([P, F2], f32, name="b2_sb")
        b3_sb = consts.tile([P, F3], f32, name="b3_sb")
        # warmup operand: memset on gpsimd (whose queue comes up
        # earliest, ~6us) so the PE can start ramping right at engine
        # bring-up. f32r is not a memset-able dtype; plain f32 warm
        # matmuls warm the PE just as well.
        warm_dt = bf16 if mode == "bf16" else f32
        warm = consts.tile([P, B], warm_dt, name="warm")
        nc.gpsimd.memset(warm, 1.0)

        # biases ride the gpsimd SWDGE: tiny, not needed until ~30us,
        # keeps the scalar queue free to fire the x chunks immediately
        # (and they queue behind the warm-tile memset above)
        nc.gpsimd.dma_start(b1_sb, b1[:, :])
        nc.gpsimd.dma_start(b2_sb, b2[:, :])
        nc.gpsimd.dma_start(b3_sb, b3[:, :])

        p_xT = tc.alloc_tile_pool(name="xT", bufs=1, side="left")
        xT_sb = p_xT.tile([P, K0, B], act_dt, name="xT_sb")
        # chunk the input load per k-tile (ACT HWDGE ring, so the
        # weight stream on the SP ring is not delayed behind it)
        for k in range(K0):
            nc.scalar.dma_start(xT_sb[:, k, :], xT[:, k, :])

        # first-block chunks: layer 1 / group 0 / kk 0 arrives as KS_L1
        # separate 128KB transfers on the SP ring (whose queue fires
        # first, ~6.8us) so the first real matmul only waits for one
        # small transfer. NB measured: routing these via the scalar
        # queue instead costs ~4us (its main-block+ACT-table preamble
        # delays the triggers); extending chunking to kk=1 also loses
        # (trigger latency pushes later block triggers out).
        wfirst = tc.alloc_tile_pool(name="wfirst", bufs=KS_L1,
                                    side="left")
        chunks = [wfirst.tile([P, FW], act_dt, name=f"wc{s}", tag="wc")
                  for s in range(KS_L1)]
        for s in range(KS_L1):
            nc.sync.dma_start(chunks[s], w1[0, 0, :, s, :])

        wpool = tc.alloc_tile_pool(
            name="w", bufs=5 if mode == "bf16" else 3, side="right")
        mmps = tc.alloc_tile_pool(name="mmpsum", bufs=8, space="PSUM")

        # HAM warmup: keep the PE busy from ~2us until the first weight
        # chunk lands, so the clock gate is 8/8 when the real stream
        # starts
        # sized so warmup ends right as the first weight chunk + x
        # chunk land (~10-12us): the PE ramp continues into real MMs
        warm_n = 7 if mode == "bf16" else 6
        wps = mmps.tile([P, B], f32, name="wps", tag="ps")
        for i in range(warm_n):
            nc.tensor.matmul(wps, warm[:, :P], warm,
                             start=(i == 0), stop=(i == warm_n - 1))

        def layer(actT, bias_sb, outT, n_k, groups, relu, wsrc,
                  store_to=None, spool=None, first_chunks=False):
            """groups: list of (fa, fgl, ksg). wsrc(gi, kk) -> dram
            block AP [P, ksg, fgl*P] (always ksg*fgl*P = KS*FW elements
            so every wt slot is one 2MB 16KB-line transfer).
            first_chunks: group 0 / kk 0 reads the pre-split chunk
            tiles instead."""
            for gi, (fa, fgl, ksg) in enumerate(groups):
                fw = fgl * P
                psums = [
                    mmps.tile([P, B], f32, name=f"ps{fa + f}", tag="ps")
                    for f in range(fgl)
                ]
                for kk in range(n_k // ksg):
                    use_chunks = first_chunks and gi == 0 and kk == 0
                    if not use_chunks:
                        wt = wpool.tile([P, ksg, fw], act_dt, name="wt",
                                        tag="wt")
                        nc.sync.dma_start(wt, wsrc(gi, kk))
                    for s in range(ksg):
                        k = kk * ksg + s
                        for f in range(fgl):
                            stat = (chunks[s][:, f * P:(f + 1) * P]
                                    if use_chunks else
                                    wt[:, s, f * P:(f + 1) * P])
                            nc.tensor.matmul(
                                psums[f],
                                stat,
                                actT[:, k, :],
                                start=(k == 0),
                                stop=(k == n_k - 1),
                            )
                if relu:
                    for f in range(fgl):
                        fi = fa + f
                        nc.scalar.activation(
                            outT[:, fi, :],
                            psums[f],
                            mybir.ActivationFunctionType.Relu,
                            bias=bias_sb[:, fi:fi + 1],
                            scale=1.0,
                        )
                else:
                    # bias-add on ScalarE (idle once the relus are done)
                    # so the store trigger on the same queue follows with
                    # no cross-engine semaphore hop
                    ot = spool.tile([P, FGL, B], f32, name="ot", tag="ot")
                    for f in range(fgl):
                        fi = fa + f
                        nc.scalar.activation(
                            ot[:, f, :],
                            psums[f],
                            mybir.ActivationFunctionType.Identity,
                            bias=bias_sb[:, fi:fi + 1],
                            scale=1.0,
                        )
                    nc.scalar.dma_start(
                        store_to[:, fa:fa + fgl, :], ot[:, :fgl, :])

        def uniform_groups(n_f, ks):
            return [(g * FGL, FGL, ks) for g in range(n_f // FGL)]

        p_h1 = tc.alloc_tile_pool(name="h1", bufs=1, side="right")
        h1T = p_h1.tile([P, K1, B], act_dt, name="h1T")
        layer(xT_sb, b1_sb, h1T, K0, uniform_groups(F1, KS_L1), True,
              lambda gi, kk: w1[kk, gi], first_chunks=True)
        wfirst.release()
        p_xT.release()

        p_h2 = tc.alloc_tile_pool(name="h2", bufs=1, side="left")
        h2T = p_h2.tile([P, K2, B], act_dt, name="h2T")
        layer(h1T, b2_sb, h2T, K1, uniform_groups(F2, KS), True,
              lambda gi, kk: w2[kk, gi])
        p_h1.release()

        # every layer-3 block stays a full 16KB-line transfer: narrow
        # f groups pack more k-tiles per block (ksg = KS*FGL/fgl)
        l3_groups = []
        fa = 0
        for fgl in L3_GROUPS:
            l3_groups.append((fa, fgl, KS * FGL // fgl))
            fa += fgl
        p_oT = tc.alloc_tile_pool(name="oT", bufs=2, side="right")
        layer(h2T, b3_sb, None, K2, l3_groups, False,
              lambda gi, kk: w3g[gi][kk],
              store_to=out, spool=p_oT)
        p_h2.release()
        mmps.release()
        p_oT.release()
        wpool.release()
        consts.release()
    nc.compile()
    return nc


def _pack_w(w: np.ndarray, np_dt, fw, ks) -> np.ndarray:
    """[d_in, d_out] -> [K/ks, d_out/fw, P, ks, fw] so one [128, ks, fw]
    DMA block reads ks*fw*itemsize bytes contiguous per partition."""
    d_in, d_out = w.shape
    K, G = d_in // P, d_out // fw
    v = w.reshape(K // ks, ks, P, G, fw)
    return np.ascontiguousarray(v.transpose(0, 3, 2, 1, 4)).astype(np_dt)


def _pack_w3_group(w: np.ndarray, np_dt, ks) -> np.ndarray:
    """[d_in, fw] -> [K/ks, P, ks, fw] (single column group)."""
    d_in, fw = w.shape
    K = d_in // P
    v = w.reshape(K // ks, ks, P, fw)
    return np.ascontiguousarray(v.transpose(0, 2, 1, 3)).astype(np_dt)


LAST_RESULT = None  # BassKernelResults of the most recent run (for test.py)


def _ensure_axon_ntff_hook():
    """Register the NTFF-profile hook that bass_utils expects under axon.
    The agent image's antenv lacks axon_hooks; synthesize it from the
    slim ctypes shim in trn_agent_boot. Only needed for trace runs."""
    import sys
    import types

    try:
        from antenv.axon_hooks import get_axon_ntff_profile_hook  # noqa: F401
        return
    except ImportError:
        pass
    try:
        import antenv
        from trn_agent_boot.trn_boot import _ntff_profile_via_ctypes

        hook = _ntff_profile_via_ctypes("/opt/axon/libaxon_pjrt.so")
        mod = types.ModuleType("antenv.axon_hooks")
        state = {"hook": hook}
        mod.get_axon_ntff_profile_hook = lambda: state["hook"]
        mod.set_axon_ntff_profile_hook = lambda h: state.update(hook=h)
        sys.modules["antenv.axon_hooks"] = mod
        antenv.axon_hooks = mod
    except Exception as e:  # degrade to untraced run
        print(f"ntff hook setup failed ({e!r}); tracing disabled")


def kernel(x, w1, b1, w2, b2, w3, b3):
    global LAST_RESULT
    os.environ.setdefault("JAX_PLATFORMS", "axon")
    mode = os.environ.get("KERNEL_MM_MODE", "bf16")
    trace = os.environ.get("KERNEL_TRACE", "0") == "1"
    if trace:
        _ensure_axon_ntff_hook()

    x = np.asarray(x, dtype=np.float32)
    b1 = np.asarray(b1, dtype=np.float32)
    b2 = np.asarray(b2, dtype=np.float32)
    b3 = np.asarray(b3, dtype=np.float32)

    w3f = np.zeros((D2, D3), dtype=np.float32)
    w3f[:, :D3_RAW] = np.asarray(w3, dtype=np.float32)
    b3f = np.zeros((D3,), dtype=np.float32)
    b3f[:D3_RAW] = b3

    np_dt = ml_dtypes.bfloat16 if mode == "bf16" else np.float32
    KS = _ks(mode)
    w1p = _pack_w(np.asarray(w1, dtype=np.float32), np_dt, FW, KS_L1)
    w2p = _pack_w(np.asarray(w2, dtype=np.float32), np_dt, FW, KS)
    w3ps = {}
    fa = 0
    for gi, fgl in enumerate(L3_GROUPS):
        fw = fgl * P
        w3ps[f"w3g{gi}"] = _pack_w3_group(
            w3f[:, fa * P:fa * P + fw], np_dt, KS * FGL // fgl)
        fa += fgl
    b1p = np.ascontiguousarray(b1.reshape(D1 // P, P).T)
    b2p = np.ascontiguousarray(b2.reshape(D2 // P, P).T)
    b3p = np.ascontiguousarray(b3f.reshape(D3 // P, P).T)

    nc = build_nc(mode=mode)
    K0 = D0 // P
    in_maps = []
    for c in range(N_CORES):
        xs = x[c * B:(c + 1) * B]  # [B, D0]
        # xT[p, k, b] = x[b, k*128 + p]
        xT = np.ascontiguousarray(
            xs.reshape(B, K0, P).transpose(2, 1, 0)).astype(np_dt)
        in_maps.append({
            "xT": xT,
            "w1": w1p, "b1": b1p,
            "w2": w2p, "b2": b2p,
            "b3": b3p,
            **w3ps,
        })

    res = run_bass_kernel_spmd(
        nc, in_maps, core_ids=list(range(N_CORES)), trace=trace
    )
    LAST_RESULT = res
    outs = []
    for r in res.results:
        oT = r["out"]  # [P, F3, B]; logits[b, fg*128+p] = oT[p, fg, b]
        outs.append(oT.transpose(2, 1, 0).reshape(B, D3))
    out = np.concatenate(outs, axis=0)
    return np.ascontiguousarray(out[:, :D3_RAW].astype(np.float32))
